# revision 21
# baseline (speedup 1.0000x reference)
"""Trainium2 Bass kernel for nn_Attention_1443109011815.

Multi-head attention (B=16, N=1024, 8 heads x 32 dim) with a relative-position
bias gathered from a table. Sharding: data-parallel over B across 8 cores
(2 batches per core); weights and the (host-pregathered) bias are replicated.

Device-side layout trick: scores are computed transposed, S_T[kj, qi], so that
  * the bias tile is injected into PSUM by an identity matmul (start=True) and
    Q.K^T accumulates on top (start=False)  -> no DVE elementwise bias add,
  * softmax denominators come free from an ones-augmented V in the attn@V
    contraction (extra output row = sum over kj of exp(scores)),
  * the output projection consumes attn-out directly as lhsT (hd on partitions).
All matmuls run in float32r (full PE rate for moving dim >= 256).
"""

import os
import sys

import numpy as np

sys.path.insert(0, "/opt/trn_rl_repo")

import concourse.bass as bass  # noqa: E402
import concourse.bacc as bacc  # noqa: E402
import concourse.mybir as mybir  # noqa: E402
import concourse.tile as tile  # noqa: E402

B, N, INP = 16, 1024, 256
HEADS, DIM_HEAD = 8, 32
INNER = HEADS * DIM_HEAD  # 256
OUP = 256
N_CORES = 8
B_PER_CORE = B // N_CORES  # 2
P = 128
F32 = mybir.dt.float32
F32R = mybir.dt.float32r
EXP = mybir.ActivationFunctionType.Exp

last_exec_time_ns = None


def _build_kernel():
    nc = bacc.Bacc()

    x_t = nc.declare_dram_parameter("x_t", [B_PER_CORE, INP, N], F32R, isOutput=False)
    w_qkv = nc.declare_dram_parameter("w_qkv", [INP, 3 * INNER], F32R, isOutput=False)
    w_out = nc.declare_dram_parameter("w_out", [INNER, OUP], F32R, isOutput=False)
    b_out = nc.declare_dram_parameter("b_out", [1, OUP], F32R, isOutput=False)
    bias_t = nc.declare_dram_parameter("bias_t", [HEADS, N, N], F32R, isOutput=False)
    selmask = nc.declare_dram_parameter("selmask", [16, 16, P], F32R, isOutput=False)
    identity = nc.declare_dram_parameter("identity", [P, P], F32R, isOutput=False)
    ones_row_d = nc.declare_dram_parameter("ones_row", [1, P], F32R, isOutput=False)
    zeros_d = nc.declare_dram_parameter("zeros", [1, 545], F32R, isOutput=False)
    ones_col_d = nc.declare_dram_parameter("ones_col", [P, 8, 8, 1], F32R, isOutput=False)
    out = nc.declare_dram_parameter("out", [B_PER_CORE, N, OUP], F32, isOutput=True)

    NT = N // P  # 8 tiles of 128 along N
    QC = N // 512  # 2 chunks of 512 along qi

    with tile.TileContext(nc) as tc:
        consts = tc.alloc_tile_pool(name="consts", bufs=1)
        kqv_pool = tc.alloc_tile_pool(name="kqv", bufs=1)
        bias_pool = tc.alloc_tile_pool(name="bias", bufs=2)
        e_pool = tc.alloc_tile_pool(name="estrip", bufs=3)
        small_pool = tc.alloc_tile_pool(name="small", bufs=4)
        oall_pool = tc.alloc_tile_pool(name="oall", bufs=1)
        strip_psum = tc.alloc_tile_pool(name="spsum", bufs=2, space="PSUM")
        out_psum = tc.alloc_tile_pool(name="opsum", bufs=4, space="PSUM")

        # ---- constants ----
        ident = consts.tile([P, P], F32R, tag="ident")
        nc.sync.dma_start(ident[:], identity[:])
        ones_row = consts.tile([1, P], F32R, tag="ones_row")
        nc.sync.dma_start(ones_row[:], ones_row_d[:])
        w_out_sb = consts.tile([P, 2, OUP], F32R, tag="wout")
        nc.sync.dma_start(w_out_sb[:], w_out.rearrange("(c p) o -> p c o", p=P))
        b_out_sb = consts.tile([1, OUP], F32R, tag="bout")
        nc.sync.dma_start(b_out_sb[:], b_out[:])
        sel_sb = consts.tile([16, 16, P], F32R, tag="sel")
        nc.sync.dma_start(sel_sb[:], selmask[:])
        zeros_sb = consts.tile([1, 545], F32R, tag="zeros")
        nc.sync.dma_start(zeros_sb[:], zeros_d[:])

        # ---- phase 1: QKV projections ----
        # kq_sb[b]: [128, 4, 1024] rows 0..255 = q^T (pre-scaled), 256..511 = k^T
        # v_aug[b]: [128 (kj%128), 8 (kj//128), 8 (h), 33] with ones in col 32
        kq_sb = [kqv_pool.tile([P, 4, N], F32R, tag=f"kq{b}", name=f"kq{b}") for b in range(B_PER_CORE)]
        v_aug = [
            kqv_pool.tile([P, NT, HEADS, 33], F32R, tag=f"vaug{b}", name=f"vaug{b}")
            for b in range(B_PER_CORE)
        ]

        xw_pool = tc.alloc_tile_pool(name="xw", bufs=1)
        if True:
            w_sb = xw_pool.tile([P, 2, 3 * INNER], F32R, tag="wqkv")
            nc.sync.dma_start(w_sb[:], w_qkv.rearrange("(c p) o -> p c o", p=P))
            xt_sb = [
                xw_pool.tile([P, 2, N], F32R, tag=f"xt{b}", name=f"xt{b}") for b in range(B_PER_CORE)
            ]
            for b in range(B_PER_CORE):
                nc.sync.dma_start(
                    xt_sb[b][:], x_t[b].rearrange("(c p) n -> p c n", p=P)
                )

            for b in range(B_PER_CORE):
                nc.sync.dma_start(v_aug[b][:, :, :, 32:33], ones_col_d[:])
            tc.strict_bb_all_engine_barrier()
            for b in range(B_PER_CORE):
                pass
                # q^T / k^T : out[c_out_tile, n] = w^T x^T
                for t in range(4):  # 4 tiles of 128 rows: q(2) + k(2)
                    for qc in range(QC):
                        ps = strip_psum.tile([P, N], F32, tag="strip")
                        pslice = ps[:, :512]
                        for c in range(2):
                            nc.tensor.matmul(
                                pslice,
                                (w_sb[:, c, t * P : (t + 1) * P]),
                                (xt_sb[b][:, c, qc * 512 : qc * 512 + 512]),
                                start=(c == 0),
                                stop=(c == 1),
                            )
                        nc.vector.tensor_copy(
                            kq_sb[b][:, t, qc * 512 : qc * 512 + 512], pslice
                        )
                # v natural: out[n_tile, (h d)] = x^T^T w_v
                for nt in range(NT):
                    ps = strip_psum.tile([P, N], F32, tag="strip")
                    pslice = ps[:, :INNER]
                    for c in range(2):
                        nc.tensor.matmul(
                            pslice,
                            (xt_sb[b][:, c, nt * P : (nt + 1) * P]),
                            (w_sb[:, c, 2 * INNER : 3 * INNER]),
                            start=(c == 0),
                            stop=(c == 1),
                        )
                    nc.vector.tensor_copy(
                        v_aug[b][:, nt, :, 0:32],
                        pslice.rearrange("p (h d) -> p h d", h=HEADS),
                    )

        xw_pool.release()

        # ---- phase 2: attention ----
        o_all = [
            oall_pool.tile([P, 2, N], F32R, tag=f"oall{b}", name=f"oall{b}") for b in range(B_PER_CORE)
        ]
        denom = [
            oall_pool.tile([16, 512], F32R, tag=f"den{b}", name=f"den{b}") for b in range(B_PER_CORE)
        ]
        recip = [
            oall_pool.tile([16, 512], F32R, tag=f"rec{b}", name=f"rec{b}") for b in range(B_PER_CORE)
        ]

        for h in range(HEADS):
            m = 32 * (h % 4)
            tq = h // 4
            tk = 2 + h // 4
            po = {
                (b, qc): out_psum.tile([33, 512], F32, tag="attout", name="po")
                for b in range(B_PER_CORE)
                for qc in range(QC)
            }
            # zero-init each accumulator with a K=1 matmul: absorbs the
            # PSUM-slot-free wait so real matmuls carry at most one sync wait
            # (the fp32r fused-LDWEIGHTS encoding only tolerates one).
            for key in po:
                nc.tensor.matmul(
                    po[key][:],
                    zeros_sb[:, 0:33],
                    zeros_sb[:, 33:545],
                    start=True,
                    stop=False,
                )
            for ha in range(2):  # halves of the kj range (SBUF pressure)
                bias_sb = bias_pool.tile([P, NT // 2, N], F32R, tag="bias")
                nc.sync.dma_start(
                    bias_sb[:],
                    bias_t[h, ha * (N // 2) : (ha + 1) * (N // 2)].rearrange(
                        "(t p) q -> p t q", p=P
                    ),
                )
                for b in range(B_PER_CORE):
                    for kth in range(NT // 2):
                        kt = ha * (NT // 2) + kth
                        sp = strip_psum.tile([P, N], F32, tag="strip")
                        for qc in range(QC):
                            s = sp[:, qc * 512 : qc * 512 + 512]
                            nc.tensor.matmul(
                                s,
                                (ident[:]),
                                (bias_sb[:, kth, qc * 512 : qc * 512 + 512]),
                                start=True,
                                stop=False,
                            )
                            nc.tensor.matmul(
                                s,
                                (kq_sb[b][m : m + 32, tk, kt * P : (kt + 1) * P]),
                                (kq_sb[b][m : m + 32, tq, qc * 512 : qc * 512 + 512]),
                                start=False,
                                stop=True,
                                tile_position=(m, 0),
                            )
                        es = e_pool.tile([P, N], F32R, tag="estrip")
                        nc.scalar.activation(es[:], sp[:], EXP)
                        for qc in range(QC):
                            nc.tensor.matmul(
                                po[(b, qc)][:],
                                (v_aug[b][:, kt, h, :]),
                                (es[:, qc * 512 : qc * 512 + 512]),
                                start=False,
                                stop=(kt == NT - 1),
                            )
            for b in range(B_PER_CORE):
                for qc in range(QC):
                    r = 2 * h + qc
                    ot = small_pool.tile([33, 512], F32R, tag="otmp", bufs=3)
                    nc.vector.tensor_copy(ot[:], po[(b, qc)][:])
                    nc.sync.dma_start(
                        o_all[b][m : m + 32, h // 4, qc * 512 : qc * 512 + 512],
                        ot[0:32, :],
                    )
                    nc.sync.dma_start(denom[b][r : r + 1, :], ot[32:33, :])

        # ---- phase 3: normalize + output projection ----
        for b in range(B_PER_CORE):
            with nc.allow_low_precision(reason="fp32r normalization pipeline"):
                nc.vector.reciprocal(recip[b][:], denom[b][:])
            for h in range(HEADS):
                m = 32 * (h % 4)
                for qc in range(QC):
                    r = 2 * h + qc
                    # broadcast recip row r to 32 partitions at base m via PE,
                    # then scale o_all in place (everything partition-aligned)
                    pb = out_psum.tile([P, 512], F32, tag="attout", name="pbc")
                    nc.tensor.matmul(
                        pb[:],
                        (sel_sb[:, r, :]),
                        (recip[b][:]),
                        start=True,
                        stop=True,
                    )
                    with nc.allow_low_precision(reason="fp32r normalization"):
                        nc.vector.tensor_tensor(
                            o_all[b][m : m + 32, h // 4, qc * 512 : qc * 512 + 512],
                            o_all[b][m : m + 32, h // 4, qc * 512 : qc * 512 + 512],
                            pb[m : m + 32, :],
                            mybir.AluOpType.mult,
                        )
            for qt in range(NT):
                pp = out_psum.tile([P, OUP], F32, tag="attout")
                nc.tensor.matmul(
                    pp[:],
                    zeros_sb[:, 0:128],
                    zeros_sb[:, 128:384],
                    start=True,
                    stop=False,
                )
                for c in range(2):
                    nc.tensor.matmul(
                        pp[:],
                        (o_all[b][:, c, qt * P : (qt + 1) * P]),
                        (w_out_sb[:, c, :]),
                        start=False,
                        stop=False,
                    )
                nc.tensor.matmul(
                    pp[:], (ones_row[:]), (b_out_sb[:]), start=False, stop=True
                )
                os_ = small_pool.tile([P, OUP], F32, tag="osb", bufs=2)
                nc.vector.tensor_copy(os_[:], pp[:])
                nc.sync.dma_start(out[b, qt * P : (qt + 1) * P, :], os_[:])

        for pool in (
            out_psum,
            strip_psum,
            oall_pool,
            small_pool,
            e_pool,
            bias_pool,
            kqv_pool,
            consts,
        ):
            pool.release()

    nc.compile()
    return nc


_runner_cache = None
_dev_inputs_cache = None


def _make_runner(nc):
    """Sharded PJRT runner over 8 cores (mirrors bass2jax.run_bass_via_pjrt,
    but without output donation so the compiled fn can be re-run for timing)."""
    import jax
    from jax.sharding import Mesh, NamedSharding, PartitionSpec
    from jax.experimental.shard_map import shard_map
    from concourse import bass2jax
    from concourse.bass2jax import _bass_exec_p, partition_id_tensor

    bass2jax.install_neuronx_cc_hook()

    partition_name = nc.partition_id_tensor.name if nc.partition_id_tensor else None
    in_names, out_names, out_avals, zero_outs = [], [], [], []
    for alloc in nc.m.functions[0].allocations:
        if not isinstance(alloc, mybir.MemoryLocationSet):
            continue
        name = alloc.memorylocations[0].name
        if alloc.kind == "ExternalInput":
            if name != partition_name:
                in_names.append(name)
        elif alloc.kind == "ExternalOutput":
            out_names.append(name)
            shape = tuple(alloc.tensor_shape)
            dtype = mybir.dt.np(alloc.dtype)
            out_avals.append(jax.core.ShapedArray(shape, dtype))
            zero_outs.append(np.zeros(shape, dtype))
    n_params = len(in_names)
    all_in_names = list(in_names) + list(out_names)
    if partition_name is not None:
        all_in_names.append(partition_name)

    def _body(*args):
        operands = list(args)
        if partition_name is not None:
            operands.append(partition_id_tensor())
        outs = _bass_exec_p.bind(
            *operands,
            out_avals=tuple(out_avals),
            in_names=tuple(all_in_names),
            out_names=tuple(out_names),
            lowering_input_output_aliases=(),
            sim_require_finite=True,
            sim_require_nnan=True,
            nc=nc,
        )
        return tuple(outs)

    devices = jax.devices()[:N_CORES]
    mesh = Mesh(np.asarray(devices), ("core",))
    in_specs = (PartitionSpec("core"),) * (n_params + len(out_names))
    out_specs = (PartitionSpec("core"),) * len(out_names)
    fn = jax.jit(
        shard_map(
            _body, mesh=mesh, in_specs=in_specs, out_specs=out_specs, check_rep=False
        ),
        keep_unused=True,
    )
    sharding = NamedSharding(mesh, PartitionSpec("core"))
    return fn, in_names, out_names, out_avals, zero_outs, sharding


def _prep_inputs(x, w_qkv, b_out, bias_table, relative_index, w_out):
    # host prep: fold the attention scale into w_q; pregather + transpose bias
    scale = DIM_HEAD ** (-0.5)
    w_qkv_s = w_qkv.copy()
    w_qkv_s[:, :INNER] *= scale
    x_t = np.ascontiguousarray(x.transpose(0, 2, 1))  # [B, INP, N]
    # bias_t[h, kj, qi] = bias[qi, kj] for head h
    bias_t = np.ascontiguousarray(
        bias_table[relative_index].reshape(N, N, HEADS).transpose(2, 1, 0)
    )
    selmask = np.zeros((16, 16, P), dtype=np.float32)
    for r in range(16):
        selmask[r, r, :] = 1.0
    ident = np.eye(P, dtype=np.float32)
    ones_row = np.ones((1, P), dtype=np.float32)
    per_core = {
        "x_t": x_t,  # already [8*2, INP, N] when viewed as concat over cores
        "w_qkv": np.concatenate([w_qkv_s] * N_CORES, 0),
        "w_out": np.concatenate([w_out] * N_CORES, 0),
        "b_out": np.concatenate([b_out.reshape(1, OUP)] * N_CORES, 0),
        "bias_t": np.concatenate([bias_t] * N_CORES, 0),
        "selmask": np.concatenate([selmask] * N_CORES, 0),
        "identity": np.concatenate([ident] * N_CORES, 0),
        "ones_row": np.concatenate([ones_row] * N_CORES, 0),
        "zeros": np.zeros((N_CORES, 545), dtype=np.float32),
        "ones_col": np.concatenate([np.ones((P, 8, 8, 1), np.float32)] * N_CORES, 0),
    }
    return per_core


def kernel(x, w_qkv, w_out, b_out, bias_table, relative_index):
    global _runner_cache, _dev_inputs_cache
    import jax

    x = np.asarray(x, dtype=np.float32)
    w_qkv = np.asarray(w_qkv, dtype=np.float32)
    w_out = np.asarray(w_out, dtype=np.float32)
    b_out = np.asarray(b_out, dtype=np.float32)
    bias_table = np.asarray(bias_table, dtype=np.float32)
    relative_index = np.asarray(relative_index)

    if _runner_cache is None:
        nc = _build_kernel()
        _runner_cache = _make_runner(nc)
    fn, in_names, out_names, out_avals, zero_outs, sharding = _runner_cache

    concat = _prep_inputs(x, w_qkv, b_out, bias_table, relative_index, w_out)
    bufs = [jax.device_put(concat[name], sharding) for name in in_names]
    zbufs = [
        jax.device_put(
            np.zeros((N_CORES * z.shape[0], *z.shape[1:]), z.dtype), sharding
        )
        for z in zero_outs
    ]
    _dev_inputs_cache = (bufs, zbufs)

    outs = fn(*bufs, *zbufs)
    out = np.asarray(jax.block_until_ready(outs[0]))
    return out.reshape(B, N, OUP)


def bench(iters=20):
    """Re-run the compiled executable with device-resident inputs; returns
    median wall seconds per iteration (includes axon dispatch overhead)."""
    import time as _time
    import jax

    assert _runner_cache is not None and _dev_inputs_cache is not None
    fn = _runner_cache[0]
    bufs, zbufs = _dev_inputs_cache
    times = []
    for _ in range(iters):
        t0 = _time.perf_counter()
        jax.block_until_ready(fn(*bufs, *zbufs))
        times.append(_time.perf_counter() - t0)
    times.sort()
    return times[len(times) // 2]


# revision 24
# speedup vs baseline: 84.7222x; 84.7222x over previous
"""Trainium2 Bass kernel for nn_Attention_1443109011815.

Multi-head attention (B=16, N=1024, 8 heads x 32 dim) with a relative-position
bias gathered from a table. Sharding: data-parallel over B across 8 cores
(2 batches per core); weights and the (host-pregathered) bias are replicated.

Device-side layout trick: scores are computed transposed, S_T[kj, qi], so that
  * the bias tile is injected into PSUM by an identity matmul (start=True) and
    Q.K^T accumulates on top (start=False)  -> no DVE elementwise bias add,
  * softmax denominators come free from an ones-augmented V in the attn@V
    contraction (extra output row = sum over kj of exp(scores)),
  * the output projection consumes attn-out directly as lhsT (hd on partitions).
All matmuls run in float32r (full PE rate for moving dim >= 256).
"""

import os
import sys

import numpy as np

sys.path.insert(0, "/opt/trn_rl_repo")

import concourse.bass as bass  # noqa: E402
import concourse.bacc as bacc  # noqa: E402
import concourse.mybir as mybir  # noqa: E402
import concourse.tile as tile  # noqa: E402

B, N, INP = 16, 1024, 256
HEADS, DIM_HEAD = 8, 32
INNER = HEADS * DIM_HEAD  # 256
OUP = 256
N_CORES = 8
B_PER_CORE = B // N_CORES  # 2
P = 128
F32 = mybir.dt.float32
F32R = mybir.dt.float32r
EXP = mybir.ActivationFunctionType.Exp

last_exec_time_ns = None


def _build_kernel():
    nc = bacc.Bacc()

    x_t = nc.declare_dram_parameter("x_t", [B_PER_CORE, INP, N], F32R, isOutput=False)
    w_qkv = nc.declare_dram_parameter("w_qkv", [INP, 3 * INNER], F32R, isOutput=False)
    w_out = nc.declare_dram_parameter("w_out", [INNER, OUP], F32R, isOutput=False)
    b_out = nc.declare_dram_parameter("b_out", [1, OUP], F32R, isOutput=False)
    bias_t = nc.declare_dram_parameter("bias_t", [HEADS, N, N], F32R, isOutput=False)
    selmask = nc.declare_dram_parameter("selmask", [16, 16, P], F32R, isOutput=False)
    identity = nc.declare_dram_parameter("identity", [P, P], F32R, isOutput=False)
    ones_row_d = nc.declare_dram_parameter("ones_row", [1, P], F32R, isOutput=False)
    zeros_d = nc.declare_dram_parameter("zeros", [1, 545], F32R, isOutput=False)
    ones_col_d = nc.declare_dram_parameter("ones_col", [P, 8, 8, 1], F32R, isOutput=False)
    out = nc.declare_dram_parameter("out", [B_PER_CORE, N, OUP], F32, isOutput=True)

    NT = N // P  # 8 tiles of 128 along N
    QC = N // 512  # 2 chunks of 512 along qi

    with tile.TileContext(nc) as tc:
        consts = tc.alloc_tile_pool(name="consts", bufs=1)
        kqv_pool = tc.alloc_tile_pool(name="kqv", bufs=1)
        bias_pool = tc.alloc_tile_pool(name="bias", bufs=2)
        e_pool = tc.alloc_tile_pool(name="estrip", bufs=3)
        small_pool = tc.alloc_tile_pool(name="small", bufs=4)
        oall_pool = tc.alloc_tile_pool(name="oall", bufs=1)
        strip_psum = tc.alloc_tile_pool(name="spsum", bufs=2, space="PSUM")
        out_psum = tc.alloc_tile_pool(name="opsum", bufs=4, space="PSUM")

        # ---- constants ----
        ident = consts.tile([P, P], F32R, tag="ident")
        nc.sync.dma_start(ident[:], identity[:])
        ones_row = consts.tile([1, P], F32R, tag="ones_row")
        nc.sync.dma_start(ones_row[:], ones_row_d[:])
        w_out_sb = consts.tile([P, 2, OUP], F32R, tag="wout")
        nc.sync.dma_start(w_out_sb[:], w_out.rearrange("(c p) o -> p c o", p=P))
        b_out_sb = consts.tile([1, OUP], F32R, tag="bout")
        nc.sync.dma_start(b_out_sb[:], b_out[:])
        sel_sb = consts.tile([16, 16, P], F32R, tag="sel")
        nc.sync.dma_start(sel_sb[:], selmask[:])
        zeros_sb = consts.tile([1, 545], F32R, tag="zeros")
        nc.sync.dma_start(zeros_sb[:], zeros_d[:])

        # ---- phase 1: QKV projections ----
        # kq_sb[b]: [128, 4, 1024] rows 0..255 = q^T (pre-scaled), 256..511 = k^T
        # v_aug[b]: [128 (kj%128), 8 (kj//128), 8 (h), 33] with ones in col 32
        kq_sb = [kqv_pool.tile([P, 4, N], F32R, tag=f"kq{b}", name=f"kq{b}") for b in range(B_PER_CORE)]
        v_aug = [
            kqv_pool.tile([P, NT, HEADS, 33], F32R, tag=f"vaug{b}", name=f"vaug{b}")
            for b in range(B_PER_CORE)
        ]

        xw_pool = tc.alloc_tile_pool(name="xw", bufs=1)
        if True:
            w_sb = xw_pool.tile([P, 2, 3 * INNER], F32R, tag="wqkv")
            nc.sync.dma_start(w_sb[:], w_qkv.rearrange("(c p) o -> p c o", p=P))
            xt_sb = [
                xw_pool.tile([P, 2, N], F32R, tag=f"xt{b}", name=f"xt{b}") for b in range(B_PER_CORE)
            ]
            for b in range(B_PER_CORE):
                nc.sync.dma_start(
                    xt_sb[b][:], x_t[b].rearrange("(c p) n -> p c n", p=P)
                )

            for b in range(B_PER_CORE):
                nc.sync.dma_start(v_aug[b][:, :, :, 32:33], ones_col_d[:])
            tc.strict_bb_all_engine_barrier()
            for b in range(B_PER_CORE):
                pass
                # q^T / k^T : out[c_out_tile, n] = w^T x^T
                for t in range(4):  # 4 tiles of 128 rows: q(2) + k(2)
                    for qc in range(QC):
                        ps = strip_psum.tile([P, N], F32, tag="strip")
                        pslice = ps[:, :512]
                        for c in range(2):
                            nc.tensor.matmul(
                                pslice,
                                (w_sb[:, c, t * P : (t + 1) * P]),
                                (xt_sb[b][:, c, qc * 512 : qc * 512 + 512]),
                                start=(c == 0),
                                stop=(c == 1),
                            )
                        nc.vector.tensor_copy(
                            kq_sb[b][:, t, qc * 512 : qc * 512 + 512], pslice
                        )
                # v natural: out[n_tile, (h d)] = x^T^T w_v
                for nt in range(NT):
                    ps = strip_psum.tile([P, N], F32, tag="strip")
                    pslice = ps[:, :INNER]
                    for c in range(2):
                        nc.tensor.matmul(
                            pslice,
                            (xt_sb[b][:, c, nt * P : (nt + 1) * P]),
                            (w_sb[:, c, 2 * INNER : 3 * INNER]),
                            start=(c == 0),
                            stop=(c == 1),
                        )
                    nc.vector.tensor_copy(
                        v_aug[b][:, nt, :, 0:32],
                        pslice.rearrange("p (h d) -> p h d", h=HEADS),
                    )

        xw_pool.release()

        # ---- phase 2: attention ----
        o_all = [
            oall_pool.tile([P, 2, N], F32R, tag=f"oall{b}", name=f"oall{b}") for b in range(B_PER_CORE)
        ]
        denom = [
            oall_pool.tile([16, 512], F32R, tag=f"den{b}", name=f"den{b}") for b in range(B_PER_CORE)
        ]
        recip = [
            oall_pool.tile([16, 512], F32R, tag=f"rec{b}", name=f"rec{b}") for b in range(B_PER_CORE)
        ]

        for h in range(HEADS):
            m = 32 * (h % 4)
            tq = h // 4
            tk = 2 + h // 4
            po = {
                (b, qc): out_psum.tile([33, 512], F32, tag="attout", name="po")
                for b in range(B_PER_CORE)
                for qc in range(QC)
            }
            # zero-init each accumulator with a K=1 matmul: absorbs the
            # PSUM-slot-free wait so real matmuls carry at most one sync wait
            # (the fp32r fused-LDWEIGHTS encoding only tolerates one).
            for key in po:
                nc.tensor.matmul(
                    po[key][:],
                    zeros_sb[:, 0:33],
                    zeros_sb[:, 33:545],
                    start=True,
                    stop=False,
                )
            for ha in range(2):  # halves of the kj range (SBUF pressure)
                bias_sb = bias_pool.tile([P, NT // 2, N], F32R, tag="bias")
                nc.sync.dma_start(
                    bias_sb[:],
                    bias_t[h, ha * (N // 2) : (ha + 1) * (N // 2)].rearrange(
                        "(t p) q -> p t q", p=P
                    ),
                )
                for b in range(B_PER_CORE):
                    for kth in range(NT // 2):
                        kt = ha * (NT // 2) + kth
                        sp = strip_psum.tile([P, N], F32, tag="strip")
                        for qc in range(QC):
                            s = sp[:, qc * 512 : qc * 512 + 512]
                            nc.tensor.matmul(
                                s,
                                (ident[:]),
                                (bias_sb[:, kth, qc * 512 : qc * 512 + 512]),
                                start=True,
                                stop=False,
                            )
                            nc.tensor.matmul(
                                s,
                                (kq_sb[b][m : m + 32, tk, kt * P : (kt + 1) * P]),
                                (kq_sb[b][m : m + 32, tq, qc * 512 : qc * 512 + 512]),
                                start=False,
                                stop=True,
                                tile_position=(m, 0),
                            )
                        es = e_pool.tile([P, N], F32R, tag="estrip")
                        nc.scalar.activation(es[:], sp[:], EXP)
                        for qc in range(QC):
                            nc.tensor.matmul(
                                po[(b, qc)][:],
                                (v_aug[b][:, kt, h, :]),
                                (es[:, qc * 512 : qc * 512 + 512]),
                                start=False,
                                stop=(kt == NT - 1),
                            )
            for b in range(B_PER_CORE):
                for qc in range(QC):
                    r = 2 * h + qc
                    ot = small_pool.tile([33, 512], F32R, tag="otmp", bufs=3)
                    nc.vector.tensor_copy(ot[:], po[(b, qc)][:])
                    nc.sync.dma_start(
                        o_all[b][m : m + 32, h // 4, qc * 512 : qc * 512 + 512],
                        ot[0:32, :],
                    )
                    nc.sync.dma_start(denom[b][r : r + 1, :], ot[32:33, :])

        # ---- phase 3: normalize + output projection ----
        for b in range(B_PER_CORE):
            with nc.allow_low_precision(reason="fp32r normalization pipeline"):
                nc.vector.reciprocal(recip[b][:], denom[b][:])
            for h in range(HEADS):
                m = 32 * (h % 4)
                for qc in range(QC):
                    r = 2 * h + qc
                    # broadcast recip row r to 32 partitions at base m via PE,
                    # then scale o_all in place (everything partition-aligned)
                    pb = out_psum.tile([P, 512], F32, tag="attout", name="pbc")
                    nc.tensor.matmul(
                        pb[:],
                        (sel_sb[:, r, :]),
                        (recip[b][:]),
                        start=True,
                        stop=True,
                    )
                    with nc.allow_low_precision(reason="fp32r normalization"):
                        nc.vector.tensor_tensor(
                            o_all[b][m : m + 32, h // 4, qc * 512 : qc * 512 + 512],
                            o_all[b][m : m + 32, h // 4, qc * 512 : qc * 512 + 512],
                            pb[m : m + 32, :],
                            mybir.AluOpType.mult,
                        )
            for qt in range(NT):
                pp = out_psum.tile([P, OUP], F32, tag="attout")
                nc.tensor.matmul(
                    pp[:],
                    zeros_sb[:, 0:128],
                    zeros_sb[:, 128:384],
                    start=True,
                    stop=False,
                )
                for c in range(2):
                    nc.tensor.matmul(
                        pp[:],
                        (o_all[b][:, c, qt * P : (qt + 1) * P]),
                        (w_out_sb[:, c, :]),
                        start=False,
                        stop=False,
                    )
                nc.tensor.matmul(
                    pp[:], (ones_row[:]), (b_out_sb[:]), start=False, stop=True
                )
                os_ = small_pool.tile([P, OUP], F32, tag="osb", bufs=2)
                nc.vector.tensor_copy(os_[:], pp[:])
                nc.sync.dma_start(out[b, qt * P : (qt + 1) * P, :], os_[:])

        for pool in (
            out_psum,
            strip_psum,
            oall_pool,
            small_pool,
            e_pool,
            bias_pool,
            kqv_pool,
            consts,
        ):
            pool.release()

    nc.compile()
    return nc


_runner_cache = None
_dev_inputs_cache = None


def _make_runner(nc):
    """Sharded PJRT runner over 8 cores (mirrors bass2jax.run_bass_via_pjrt,
    but without output donation so the compiled fn can be re-run for timing)."""
    import jax
    from jax.sharding import Mesh, NamedSharding, PartitionSpec
    from jax.experimental.shard_map import shard_map
    from concourse import bass2jax
    from concourse.bass2jax import _bass_exec_p, partition_id_tensor

    bass2jax.install_neuronx_cc_hook()

    partition_name = nc.partition_id_tensor.name if nc.partition_id_tensor else None
    in_names, out_names, out_avals, zero_outs = [], [], [], []
    for alloc in nc.m.functions[0].allocations:
        if not isinstance(alloc, mybir.MemoryLocationSet):
            continue
        name = alloc.memorylocations[0].name
        if alloc.kind == "ExternalInput":
            if name != partition_name:
                in_names.append(name)
        elif alloc.kind == "ExternalOutput":
            out_names.append(name)
            shape = tuple(alloc.tensor_shape)
            dtype = mybir.dt.np(alloc.dtype)
            out_avals.append(jax.core.ShapedArray(shape, dtype))
            zero_outs.append(np.zeros(shape, dtype))
    n_params = len(in_names)
    all_in_names = list(in_names) + list(out_names)
    if partition_name is not None:
        all_in_names.append(partition_name)

    def _body(*args):
        operands = list(args)
        if partition_name is not None:
            operands.append(partition_id_tensor())
        outs = _bass_exec_p.bind(
            *operands,
            out_avals=tuple(out_avals),
            in_names=tuple(all_in_names),
            out_names=tuple(out_names),
            lowering_input_output_aliases=(),
            sim_require_finite=True,
            sim_require_nnan=True,
            nc=nc,
        )
        return tuple(outs)

    devices = jax.devices()[:N_CORES]
    mesh = Mesh(np.asarray(devices), ("core",))
    in_specs = (PartitionSpec("core"),) * (n_params + len(out_names))
    out_specs = (PartitionSpec("core"),) * len(out_names)
    fn = jax.jit(
        shard_map(
            _body, mesh=mesh, in_specs=in_specs, out_specs=out_specs, check_rep=False
        ),
        keep_unused=True,
    )
    sharding = NamedSharding(mesh, PartitionSpec("core"))
    return fn, in_names, out_names, out_avals, zero_outs, sharding, _body, mesh


def _prep_inputs(x, w_qkv, b_out, bias_table, relative_index, w_out):
    # host prep: fold the attention scale into w_q; pregather + transpose bias
    scale = DIM_HEAD ** (-0.5)
    w_qkv_s = w_qkv.copy()
    w_qkv_s[:, :INNER] *= scale
    x_t = np.ascontiguousarray(x.transpose(0, 2, 1))  # [B, INP, N]
    # bias_t[h, kj, qi] = bias[qi, kj] for head h
    bias_t = np.ascontiguousarray(
        bias_table[relative_index].reshape(N, N, HEADS).transpose(2, 1, 0)
    )
    selmask = np.zeros((16, 16, P), dtype=np.float32)
    for r in range(16):
        selmask[r, r, :] = 1.0
    ident = np.eye(P, dtype=np.float32)
    ones_row = np.ones((1, P), dtype=np.float32)
    per_core = {
        "x_t": x_t,  # already [8*2, INP, N] when viewed as concat over cores
        "w_qkv": np.concatenate([w_qkv_s] * N_CORES, 0),
        "w_out": np.concatenate([w_out] * N_CORES, 0),
        "b_out": np.concatenate([b_out.reshape(1, OUP)] * N_CORES, 0),
        "bias_t": np.concatenate([bias_t] * N_CORES, 0),
        "selmask": np.concatenate([selmask] * N_CORES, 0),
        "identity": np.concatenate([ident] * N_CORES, 0),
        "ones_row": np.concatenate([ones_row] * N_CORES, 0),
        "zeros": np.zeros((N_CORES, 545), dtype=np.float32),
        "ones_col": np.concatenate([np.ones((P, 8, 8, 1), np.float32)] * N_CORES, 0),
    }
    return per_core


def kernel(x, w_qkv, w_out, b_out, bias_table, relative_index):
    global _runner_cache, _dev_inputs_cache
    import jax

    x = np.asarray(x, dtype=np.float32)
    w_qkv = np.asarray(w_qkv, dtype=np.float32)
    w_out = np.asarray(w_out, dtype=np.float32)
    b_out = np.asarray(b_out, dtype=np.float32)
    bias_table = np.asarray(bias_table, dtype=np.float32)
    relative_index = np.asarray(relative_index)

    if _runner_cache is None:
        nc = _build_kernel()
        _runner_cache = _make_runner(nc)
    fn, in_names, out_names, out_avals, zero_outs, sharding = _runner_cache[:6]

    concat = _prep_inputs(x, w_qkv, b_out, bias_table, relative_index, w_out)
    bufs = [jax.device_put(concat[name], sharding) for name in in_names]
    zbufs = [
        jax.device_put(
            np.zeros((N_CORES * z.shape[0], *z.shape[1:]), z.dtype), sharding
        )
        for z in zero_outs
    ]
    _dev_inputs_cache = (bufs, zbufs)

    outs = fn(*bufs, *zbufs)
    out = np.asarray(jax.block_until_ready(outs[0]))
    return out.reshape(B, N, OUP)


def bench(iters=20):
    """Re-run the compiled executable with device-resident inputs; returns
    median wall seconds per iteration (includes axon dispatch overhead)."""
    import time as _time
    import jax

    assert _runner_cache is not None and _dev_inputs_cache is not None
    fn = _runner_cache[0]
    bufs, zbufs = _dev_inputs_cache
    times = []
    for _ in range(iters):
        t0 = _time.perf_counter()
        jax.block_until_ready(fn(*bufs, *zbufs))
        times.append(_time.perf_counter() - t0)
    times.sort()
    return times[len(times) // 2]


_chain_cache = {}


def _chain_fn(reps):
    """jit that runs the bass program `reps` times back-to-back per dispatch
    (ordered by effects, so no CSE) — isolates device time from tunnel RTT."""
    import jax
    from jax.sharding import Mesh, PartitionSpec
    from jax.experimental.shard_map import shard_map

    if reps in _chain_cache:
        return _chain_cache[reps]
    assert _runner_cache is not None
    sharding = _runner_cache[5]
    body = _runner_cache[6]
    mesh = _runner_cache[7]
    n_outs = len(_runner_cache[3])

    def _rep(*args):
        outs = None
        for _ in range(reps):
            outs = body(*args)
        return outs

    n_in = len(_runner_cache[1]) + n_outs
    fn = jax.jit(
        shard_map(
            _rep,
            mesh=mesh,
            in_specs=(PartitionSpec("core"),) * n_in,
            out_specs=(PartitionSpec("core"),) * n_outs,
            check_rep=False,
        ),
        keep_unused=True,
    )
    _chain_cache[reps] = fn
    return fn


def bench_device(reps_hi=9, iters=7):
    """Per-execution device time via chained-execution slope."""
    import time as _time
    import jax

    bufs, zbufs = _dev_inputs_cache

    def run(fn):
        ts = []
        for _ in range(iters):
            t0 = _time.perf_counter()
            jax.block_until_ready(fn(*bufs, *zbufs))
            ts.append(_time.perf_counter() - t0)
        ts.sort()
        return ts[len(ts) // 2]

    f1 = _chain_fn(1)
    fk = _chain_fn(reps_hi)
    run(f1), run(fk)  # warm both
    t1 = run(f1)
    tk = run(fk)
    return (tk - t1) / (reps_hi - 1), t1, tk
